# revision 62
# baseline (speedup 1.0000x reference)
"""Trainium2 Bass kernel for nn_Attention_78812649881818.

reference:
    attn = softmax(output @ context^T, axis=-1)        # [B, O, I]
    mix  = attn @ context                              # [B, O, D]
    out  = tanh(concat([mix, output], -1) @ W + b)     # [B, O, D]
    returns (out, attn)

Sharding: data-parallel over batch, 2 batches per core on 8 NeuronCores.

Per-core dataflow (per batch):
  - interleaved 1MB cast-DMA loads round inputs to float32r (TF32-like,
    ~11 mantissa bits, full PE rate at N>=256); K natural doubles as V
  - PE transposes (fp32r, exact) build Q^T [d,o] and K^T [d,i]
  - per o-tile: S = Q K^T accumulated over d into two [128,1024] PSUM slots;
    the softmax shift uses the max of only the FIRST half of the row (any
    shift cancels in normalization; overflow would need a >88-nat gap
    between half-row maxes, impossible for N(0,~22) logits) so exp of the
    first half runs on ACT during the second half's S matmuls
  - P^T transposes read unnormalized P so the gpsimd normalize + attn store
    stay off the PE critical path; 1/rowsum is folded into the linear
  - mix_un^T = V-stationary matmuls with P^T moving (dk-pair PSUM passes)
  - linear: mix_un^T@W1 and Q^T@W2 + ones x b in one PSUM slot; combine
    pre = acc_m*recip + acc_q on DVE; tanh on ACT from PSUM; linears are
    emitted interleaved into the next half's S loop / next batch's
    transpose phase so the scheduler fills softmax-latency gaps
  - walrus here rejects >1 semaphore wait per instruction; a post-pass
    hoists extra waits onto same-engine NoOps
"""
import numpy as np

import concourse.bass as bass
import concourse.mybir as mybir
import concourse.tile as tile
from concourse.bass_utils import run_bass_kernel_spmd
from concourse.masks import make_identity

f32 = mybir.dt.float32
f32r = mybir.dt.float32r
AF = mybir.ActivationFunctionType
ALU = mybir.AluOpType

B, OUT_LEN, IN_LEN, DIM = 16, 1024, 2048, 512
N_CORES = 8
B_LOC = B // N_CORES  # batches per core
N_OT = OUT_LEN // 128     # 8 o-tiles
N_IT = IN_LEN // 128      # 16 i-tiles
N_DK = DIM // 128         # 4 d-tiles
CDIM = 2 * DIM            # 1024 (concat feature dim)


def _split_excess_waits(nc):
    """walrus on this stack rejects instructions carrying >1 semaphore wait
    ("Too many sync wait commands"); hoist extras onto NoOps on the same
    engine immediately before the offending instruction."""
    uid = 0
    for f in nc.m.functions:
        for blk in f.blocks:
            new = []
            for ins in blk.instructions:
                si = ins.sync_info
                if si is not None and si.on_wait and len(si.on_wait) > 1:
                    waits = list(si.on_wait)
                    while len(waits) > 1:
                        nop = mybir.InstNoOp(name=f"I-wsplit-{uid}")
                        uid += 1
                        nop.engine = ins.engine
                        nop.sync_info = mybir.SyncInfo(
                            on_wait=[waits.pop(0)], on_update=[]
                        )
                        new.append(nop)
                    ins.sync_info = mybir.SyncInfo(
                        on_wait=waits, on_update=list(si.on_update)
                    )
                new.append(ins)
            blk.instructions = new


def _build(fixup: bool = True):
    nc = bass.Bass()

    q_in = nc.declare_dram_parameter("q", [B_LOC, OUT_LEN, DIM], f32, isOutput=False)
    k_in = nc.declare_dram_parameter("k", [B_LOC, IN_LEN, DIM], f32, isOutput=False)
    w_in = nc.declare_dram_parameter("w", [CDIM, DIM], f32, isOutput=False)
    b_in = nc.declare_dram_parameter("bias", [1, DIM], f32, isOutput=False)
    o_out = nc.declare_dram_parameter("out", [B_LOC, OUT_LEN, DIM], f32, isOutput=True)
    a_out = nc.declare_dram_parameter(
        "attn", [B_LOC, OUT_LEN, IN_LEN], f32, isOutput=True
    )

    with tile.TileContext(nc) as tc:
        with (
            tc.tile_pool(name="const", bufs=1) as cpool,
            tc.tile_pool(name="perm", bufs=1) as perm,
            tc.tile_pool(name="perm2", bufs=1) as perm2,
            tc.tile_pool(name="pairp", bufs=1) as pairp,
            tc.tile_pool(name="work", bufs=2) as work,
            tc.tile_pool(name="punp", bufs=4) as punp,
            tc.tile_pool(name="nat", bufs=1) as nat,
            tc.tile_pool(name="small", bufs=8) as small,
            tc.tile_pool(name="spsum", bufs=3, space="PSUM") as spsum,
            tc.tile_pool(name="tpsum", bufs=2, space="PSUM") as tpsum,
        ):
            # ---- constants ----
            ident = cpool.tile([128, 128], f32)
            make_identity(nc, ident[:])
            ident_r = cpool.tile([128, 128], f32r)
            nc.vector.tensor_copy(ident_r[:], ident[:])
            ones_f = cpool.tile([1, 128], f32)
            nc.vector.memset(ones_f[:], 1.0)
            ones_r = cpool.tile([1, 128], f32r)
            nc.vector.tensor_copy(ones_r[:], ones_f[:])
            zero_b = cpool.tile([128, 1], f32)
            nc.vector.memset(zero_b[:], 0.0)
            b_sb = cpool.tile([1, DIM], f32r)
            w_all = cpool.tile([128, 8 * DIM], f32r)  # [ck(8), j(512)]

            b_bc = cpool.tile([128, DIM], f32)

            copy_engines = [nc.vector, nc.scalar]
            cp_i = 0

            def copy(dst, src):
                nonlocal cp_i
                eng = copy_engines[cp_i % 2]
                cp_i += 1
                if eng is nc.scalar:
                    nc.scalar.activation(dst, src, AF.Copy)
                else:
                    nc.vector.tensor_copy(dst, src)

            pending_linears = []

            def emit_linear(e_lb, e_ot, e_rot, e_mixT, e_recip, e_qT):
                pl = spsum.tile([128, 1024], f32, tag="s_psum")
                acc_m = pl[:, 0:512]   # mix_un @ W1 (needs recip scale)
                acc_q = pl[:, 512:1024]  # q @ W2 + 1 x b
                for dk in range(N_DK):
                    nc.tensor.matmul(
                        acc_m,
                        e_mixT[:, dk * 512 + e_rot * 128:
                               dk * 512 + (e_rot + 1) * 128],
                        w_all[:, dk * DIM:(dk + 1) * DIM],
                        start=(dk == 0),
                        stop=(dk == N_DK - 1),
                    )
                for dk in range(N_DK):
                    nc.tensor.matmul(
                        acc_q,
                        e_qT[e_ot // 2][:, dk * 256 + (e_ot % 2) * 128:
                                       dk * 256 + (e_ot % 2 + 1) * 128],
                        w_all[:, (4 + dk) * DIM:(5 + dk) * DIM],
                        start=(dk == 0),
                        stop=(dk == N_DK - 1),
                    )
                # pre = acc_m * recip + (acc_q + b); DVE reads one PSUM operand
                q_sb = work.tile([128, DIM], f32, tag="q_sb")
                nc.vector.tensor_tensor(
                    out=q_sb[:], in0=acc_q, in1=b_bc[:], op=ALU.add
                )
                nc.vector.scalar_tensor_tensor(
                    out=acc_m, in0=acc_m, scalar=e_recip[:],
                    in1=q_sb[:], op0=ALU.mult, op1=ALU.add,
                )
                out_sb = work.tile([128, DIM], f32, tag="out_sb")
                nc.scalar.activation(out_sb[:], acc_m, AF.Tanh, bias=zero_b[:])
                nc.sync.dma_start(
                    o_out[e_lb, e_ot * 128:(e_ot + 1) * 128, :], out_sb[:]
                )

            for lb in range(B_LOC):
                recips = []
                # ---- persistent per-batch tiles ----
                qT_g = []
                for g in range(4):
                    qt = perm2.tile([128, N_DK * 256], f32r, tag=f"qT{g}")
                    qT_g.append(qt)  # [dk, o_local(256)] for o-tiles 2g,2g+1
                kTg = []
                for g in range(4):
                    kt = perm.tile([128, N_DK * 512], f32r, tag=f"kT{g}")
                    kTg.append(kt)  # [dk, i_local(512)] for i-tiles 4g..4g+3
                v_g = []
                for g in range(4):
                    vt = perm.tile([128, 4 * DIM], f32r, tag=f"v{g}")
                    v_g.append(vt)  # [it_local(4), d] for i-tiles 4g..4g+3


                # ---- interleaved 1MB cast-DMA loads (fills both SWDGE queues) ----
                q_nats = []
                for g in range(2):
                    qn = nat.tile([128, 4 * DIM], f32r, tag=f"q_nat{g}")
                    q_nats.append(qn)
                order = [("q", 0), ("k", 0), ("q", 1), ("k", 1), ("k", 2), ("k", 3)]
                for kind, g in order:
                    if kind == "q":
                        nc.gpsimd.dma_start(
                            q_nats[g][:].rearrange("p (t d) -> p t d", t=4),
                            q_in[lb, g * 512:(g + 1) * 512, :]
                            .rearrange("(t p) d -> p t d", p=128),
                        )
                    else:
                        nc.gpsimd.dma_start(
                            v_g[g][:].rearrange("p (t d) -> p t d", t=4),
                            k_in[lb, g * 512:(g + 1) * 512, :]
                            .rearrange("(t p) d -> p t d", p=128),
                        )

                # ---- build Q^T ----
                for g in range(2):
                    for t in range(4):
                        ot = g * 4 + t
                        if pending_linears:
                            emit_linear(*pending_linears.pop(0))
                        ps = tpsum.tile([128, 512], f32r, tag="tstage")
                        for dk in range(N_DK):
                            nc.tensor.transpose(
                                ps[:, dk * 128:(dk + 1) * 128],
                                q_nats[g][:, t * DIM + dk * 128:
                                           t * DIM + (dk + 1) * 128],
                                ident_r[:],
                            )
                        copy(
                            qT_g[ot // 2][:]
                            .rearrange("p (dk o) -> p dk o", dk=N_DK)
                            [:, :, (ot % 2) * 128:(ot % 2 + 1) * 128],
                            ps[:, 0:512].rearrange("p (dk o) -> p dk o", dk=N_DK),
                        )

                # ---- build K^T ----
                for g in range(4):
                    for t in range(4):
                        it = g * 4 + t
                        ps = tpsum.tile([128, 512], f32r, tag="tstage")
                        for dk in range(N_DK):
                            nc.tensor.transpose(
                                ps[:, dk * 128:(dk + 1) * 128],
                                v_g[g][:, t * DIM + dk * 128:
                                       t * DIM + (dk + 1) * 128],
                                ident_r[:],
                            )
                        copy(
                            kTg[g][:].rearrange("p (dk i) -> p dk i", dk=N_DK)
                            [:, :, t * 128:(t + 1) * 128],
                            ps[:, 0:512].rearrange("p (dk i) -> p dk i", dk=N_DK),
                        )


                if lb == 0:
                    nc.gpsimd.dma_start(
                        w_all[:].rearrange("p (c d) -> p c d", c=8),
                        w_in[:].rearrange("(c p) d -> p c d", p=128),
                    )
                    nc.gpsimd.dma_start(b_sb[:], b_in[:])
                    ps_b = tpsum.tile([128, 512], f32, tag="tstage")
                    nc.tensor.matmul(
                        ps_b[:], ones_r[:], b_sb[:], start=True, stop=True
                    )
                    nc.scalar.activation(b_bc[:], ps_b[:], AF.Copy)

                for half in range(2):
                    pT_g = []
                    for g in range(4):
                        pt = pairp.tile([128, 4 * 512], f32r, tag=f"pT{g}")
                        pT_g.append(pt)  # [it_local(4), o(512)] for i-tiles 4g..4g+3

                    for rot in range(4):
                        ot = half * 4 + rot
                        if pending_linears:
                            emit_linear(*pending_linears.pop(0))
                        # ---- S = Q K^T for this o-tile ----
                        # softmax shift from chunks 0-1 only: any shift is
                        # cancelled by normalization; overflow would need a
                        # >88-nat gap between chunk maxes (impossible for
                        # N(0,~22) logits). This lets expA overlap the second
                        # half of the S matmuls.
                        psA = spsum.tile([128, 1024], f32, tag="s_psum")
                        psB = spsum.tile([128, 1024], f32, tag="s_psum")
                        mx2 = small.tile([128, 2], f32, tag="mx2")
                        neg_mx = small.tile([128, 1], f32, tag="neg_mx")
                        p_un = punp.tile([128, IN_LEN], f32r, tag="p_un")
                        accA = small.tile([128, 1], f32, tag="accA")
                        accB = small.tile([128, 1], f32, tag="accB")
                        for ic in range(4):
                            tgt = (psA if ic < 2 else psB)[
                                :, (ic % 2) * 512:(ic % 2 + 1) * 512
                            ]
                            for dk in range(N_DK):
                                nc.tensor.matmul(
                                    tgt,
                                    qT_g[ot // 2][:, dk * 256 + (ot % 2) * 128:
                                                  dk * 256 + (ot % 2 + 1) * 128],
                                    kTg[ic][:, dk * 512:(dk + 1) * 512],
                                    start=(dk == 0),
                                    stop=(dk == N_DK - 1),
                                )
                            if ic < 2:
                                # row max of this chunk while later chunks run
                                nc.vector.reduce_max(
                                    mx2[:, ic:ic + 1], tgt,
                                    axis=mybir.AxisListType.X,
                                )
                            if ic == 1:
                                nc.vector.tensor_reduce(
                                    neg_mx[:], mx2[:], axis=mybir.AxisListType.X,
                                    op=ALU.max, negate=True,
                                )
                                # expA overlaps S chunks 2-3 on the PE
                                nc.scalar.activation(
                                    p_un[:, 0:1024], psA[:], AF.Exp,
                                    bias=neg_mx[:], accum_out=accA[:],
                                )
                        nc.scalar.activation(
                            p_un[:, 1024:2048], psB[:], AF.Exp,
                            bias=neg_mx[:], accum_out=accB[:],
                        )
                        recip = small.tile([128, 1], f32, tag="recip")
                        nc.vector.tensor_tensor(
                            out=recip[:], in0=accA[:], in1=accB[:], op=ALU.add
                        )
                        nc.vector.reciprocal(recip[:], recip[:])
                        recips.append(recip)
                        # ---- P^T ----
                        for g in range(4):
                            ps = tpsum.tile([128, 512], f32r, tag="tstage")
                            for j in range(4):
                                it = g * 4 + j
                                nc.tensor.transpose(
                                    ps[:, j * 128:(j + 1) * 128],
                                    p_un[:, it * 128:(it + 1) * 128],
                                    ident_r[:],
                                )
                            copy(
                                pT_g[g][:].rearrange("p (it o) -> p it o", it=4)
                                [:, :, rot * 128:(rot + 1) * 128],
                                ps[:].rearrange("p (it o) -> p it o", it=4),
                            )
                        # normalize in place (after transposes) and store attn
                        nc.gpsimd.tensor_scalar(
                            p_un[:], p_un[:].bitcast(f32), recip[:], None, ALU.mult
                        )
                        nc.sync.dma_start(
                            a_out[lb, ot * 128:(ot + 1) * 128, :],
                            p_un[:].bitcast(f32),
                        )

                    # ---- mix^T for this half: two single-slot dk passes ----
                    mixT = work.tile([128, 2048], f32r, tag="mixT")
                    for dp in range(2):
                        mix_ps = spsum.tile([128, 1024], f32, tag="s_psum")
                        for dh in range(2):
                            dk = dp * 2 + dh
                            tgt = mix_ps[:, dh * 512:(dh + 1) * 512]
                            for it in range(N_IT):
                                nc.tensor.matmul(
                                    tgt,
                                    v_g[it // 4][:, (it % 4) * DIM + dk * 128:
                                                 (it % 4) * DIM + (dk + 1) * 128],
                                    pT_g[it // 4][:, (it % 4) * 512:
                                                  (it % 4 + 1) * 512],
                                    start=(it == 0),
                                    stop=(it == N_IT - 1),
                                )
                        copy(mixT[:, dp * 1024:(dp + 1) * 1024], mix_ps[:])

                    # ---- queue linears; emitted interleaved with next half's S ----
                    for rot in range(4):
                        pending_linears.append(
                            (lb, half * 4 + rot, rot, mixT, recips[half * 4 + rot], qT_g)
                        )

            while pending_linears:
                emit_linear(*pending_linears.pop(0))

    if fixup:
        _split_excess_waits(nc)
    return nc


_CACHED = None


def kernel(output, context, W, b):
    global _CACHED
    if _CACHED is None:
        _CACHED = _build()
    nc = _CACHED

    output = np.ascontiguousarray(output, dtype=np.float32)
    context = np.ascontiguousarray(context, dtype=np.float32)
    W = np.ascontiguousarray(W, dtype=np.float32)
    b2 = np.ascontiguousarray(b, dtype=np.float32).reshape(1, DIM)

    in_maps = [
        {
            "q": output[c * B_LOC:(c + 1) * B_LOC],
            "k": context[c * B_LOC:(c + 1) * B_LOC],
            "w": W,
            "bias": b2,
        }
        for c in range(N_CORES)
    ]
    res = run_bass_kernel_spmd(nc, in_maps, list(range(N_CORES)))
    out = np.concatenate([res.results[c]["out"] for c in range(N_CORES)], axis=0)
    attn = np.concatenate([res.results[c]["attn"] for c in range(N_CORES)], axis=0)
    return out, attn
